# revision 44
# baseline (speedup 1.0000x reference)
"""Cross multi-headed attention with relative position bias, sharded over 8 trn2 cores.

Sharding: query positions (T1=1024) split 128/core. pos_k (the 256MB tensor) is
indexed by query position, so each core streams a disjoint 1/8 slice exactly
once. K/V are computed from the full x on every core (cheap); outputs are
disjoint q-slices gathered on host.

Layout tricks:
  - pos_k is host-transposed/packed to (qgroup, dk+B, 2*T2) so it streams as
    the matmul moving operand (contraction dim dk on partitions) in 294KB DMAs,
    self-paced on the SP engine (a dedicated queue that blocks on pool bufs).
  - The mask penalty is folded into the Bmat matmul as 8 extra contraction
    rows (one-hot per batch x -1e4 penalty): masking costs zero vector ops and
    masked attn weights are exactly 0 after exp underflow.
  - Bmat comes out of its matmul in per-q tiles (32 bh-rows x T2); an
    SBUF->SBUF permute DMA regroups it into a resident 8MB SBUF buffer of
    per-(b,h) tiles (128q x T2) - no DRAM roundtrip - which an identity-matmul
    accumulates straight onto the QK^T scores in PSUM.
  - xn^T lives in a small ring (consumed per-chunk by the K/V projections),
    freeing the SBUF needed for the resident Bmat buffer.
  - LayerNorm gain/bias are folded into the projection weights on host;
    stats come from one bn_stats/bn_aggr pass; mean/rstd applied in natural
    layout (GPSIMD), then xn is PE-transposed for the projections.
  - Softmax denominators come from the exp activation's accum_out for free;
    no max-subtraction needed since scores are bounded (~N(0,2) after scale).
"""

import numpy as np
import ml_dtypes
from contextlib import ExitStack

import concourse.bass as bass
import concourse.tile as tile
import concourse.mybir as mybir
from concourse import bacc
from concourse.bass_utils import run_bass_kernel_spmd
from concourse.masks import make_identity

F32 = mybir.dt.float32
F32R = mybir.dt.float32r
BF16 = mybir.dt.bfloat16
FP8 = mybir.dt.float8e3

B, T, F, H, DK = 8, 1024, 256, 4, 64
NCORES = 8
TQ = T // NCORES          # query rows per core (128)
TOK = B * T               # tokens for K/V (8192)
TOKQ = B * TQ             # query tokens per core (1024)
NBH = B * H               # 32 (b,h) pairs
KAUG = DK + B             # bmat contraction rows: 64 qk + 8 mask-penalty
EPS = 1e-5
PEN = 1e4                 # mask penalty (score -= PEN/8 => exp underflows to 0)

CFG = dict()

AF = mybir.ActivationFunctionType
ALU = mybir.AluOpType
AXX = mybir.AxisListType


def build_program(cfg=CFG, bv2_nz=False, bo_nz=False):
    nc = bacc.Bacc("TRN2", target_bir_lowering=False, debug=False)

    # ---- DRAM I/O ----
    # x packed as 16 super-tiles: [st, tok%128, (tok//128 % 4)*F + f]
    x_d = nc.dram_tensor("xp", [TOK // 512, 128, 4 * F], BF16, kind="ExternalInput")
    qt_d = nc.dram_tensor("q_t", [F, TOKQ], BF16, kind="ExternalInput")
    # pos_k packed 4 q per tile on all 128 partitions (4KB lines):
    # rows 0:64 = d-rows of even local q, 64:128 = odd; cols 0:T = q-pair A,
    # T:2T = q-pair B.  [g, band*64+d, pair*T + t]
    posk_d = nc.dram_tensor("posk_p", [TQ // 4, 128, 2 * T], BF16,
                            kind="ExternalInput")
    # mask penalty operand: tile m covers groups 4m..4m+3 (16 q); row
    # s*32 + j*8 + b = (1-mask[b, q=16m+4s+j, t])
    maskp_d = nc.dram_tensor("mask_p", [TQ // 16, 128, T], BF16,
                             kind="ExternalInput")
    # stationary for the mask matmul: [s*32+j*8+b, j*32+b*H+h] = -PEN
    onehp_d = nc.dram_tensor("onehp", [128, 128], BF16, kind="ExternalInput")
    # DRAM bounce for the Bmat regroup: written per-group (contiguous, all 16
    # DMA ports), read back per-8-group batch into contiguous 32-partition
    # blocks of bmall (8 ports) - avoids the 4-partition SBUF->SBUF scatter.
    bounce_d = nc.dram_tensor("bm_bounce", [TQ // 4, 128, T], BF16,
                              kind="Internal")
    wq_d = nc.dram_tensor("wq", [F, F], BF16, kind="ExternalInput")
    wk_d = nc.dram_tensor("wk2", [F, F], BF16, kind="ExternalInput")
    wv_d = nc.dram_tensor("wv2", [F, F], BF16, kind="ExternalInput")
    wo_d = nc.dram_tensor("wo", [F, F], F32R, kind="ExternalInput")
    bq_d = nc.dram_tensor("bq_cols", [128, 2], F32, kind="ExternalInput")
    bk_d = nc.dram_tensor("bk2_cols", [128, 2], F32, kind="ExternalInput")
    bv_d = nc.dram_tensor("bv2_row", [1, F], BF16, kind="ExternalInput")
    bo_d = nc.dram_tensor("bo_row", [1, F], F32R, kind="ExternalInput")
    ones_d = nc.dram_tensor("ones_row", [1, 128], F32R, kind="ExternalInput")
    out_d = nc.dram_tensor("out", [B, TQ, F], F32, kind="ExternalOutput")

    with tile.TileContext(nc) as tc, ExitStack() as ctx:
        consts = ctx.enter_context(tc.tile_pool(name="consts", bufs=1))
        persist = ctx.enter_context(tc.tile_pool(name="persist", bufs=1))

        # ---- constants (loaded on sync before the posk stream starts) ----
        id_bf = consts.tile([128, 128], BF16)
        make_identity(nc, id_bf)
        id_f32 = consts.tile([128, 128], F32)
        make_identity(nc, id_f32)
        wq_sb = consts.tile([128, 2 * F], BF16)   # [f%128, (f//128)*F + n]
        wk_sb = consts.tile([128, 2 * F], BF16)
        wv_sb = consts.tile([128, 2 * F], BF16)
        wo_sb = consts.tile([64, 4 * F], F32R)    # [hd%64, (hd//64)*F + n]
        onehp_sb = consts.tile([128, 128], BF16)
        nc.sync.dma_start(onehp_sb[:], onehp_d[:])
        zrow = consts.tile([1, 512], BF16)
        nc.gpsimd.memset(zrow[:], 0.0)
        for w_sb, w_d in ((wq_sb, wq_d), (wk_sb, wk_d), (wv_sb, wv_d)):
            for fc in range(2):
                nc.sync.dma_start(w_sb[:, fc * F:(fc + 1) * F],
                                  w_d[fc * 128:(fc + 1) * 128, :])
        for ci in range(4):
            nc.sync.dma_start(wo_sb[:, ci * F:(ci + 1) * F],
                              wo_d[ci * 64:(ci + 1) * 64, :])
        bq_sb = consts.tile([128, 2], F32)
        nc.sync.dma_start(bq_sb[:], bq_d[:])
        bk_sb = consts.tile([128, 2], F32)
        nc.sync.dma_start(bk_sb[:], bk_d[:])
        if bv2_nz:
            bv_sb = consts.tile([1, F], BF16)
            nc.sync.dma_start(bv_sb[:], bv_d[:])
            ones_bf = consts.tile([1, 128], BF16)
            nc.gpsimd.memset(ones_bf[:], 1.0)
        if bo_nz:
            bo_sb = consts.tile([1, F], F32R)
            nc.sync.dma_start(bo_sb[:], bo_d[:])
            ones_r = consts.tile([1, 128], F32R)
            nc.sync.dma_start(ones_r[:], ones_d[:])
        eps_col = consts.tile([128, 1], F32)
        nc.gpsimd.memset(eps_col[:], EPS)

        # ---- persistent activations ----
        kt_sb = persist.tile([128, 2 * TOK], BF16)     # K^T: [n%128, (n//128)*TOK+tok]
        v_sb = persist.tile([128, (TOK // 128) * F], BF16)  # V: [tok%128,(tok//128)*F+hd]
        # rows 0:64 = Q^T [d, b*H*TQ + h*TQ + q]; rows 64:128 = copy of 0:64
        qq2 = persist.tile([128, TQ * NBH], BF16)
        # resident regrouped Bmat: [q, bh*T + t]  (8MB, 64KB/partition)
        bmall = persist.tile([128, NBH * T], BF16)

        NST = TOK // 512  # 16 super tiles

        # ================= Phase B: Q^T projection -> qq_sb =================
        with tc.tile_pool(name="qstage", bufs=1) as qstage, \
             tc.tile_pool(name="ps_q", bufs=2, space="PSUM") as ps_q:
            qt_tiles = []
            for fc in range(2):
                qt = qstage.tile([128, TOKQ], BF16, tag=f"qt{fc}")
                nc.scalar.dma_start(qt[:], qt_d[fc * 128:(fc + 1) * 128, :])
                qt_tiles.append(qt)
            qv = qq2[0:DK].rearrange("p (b h q) -> p b q h", h=H, q=TQ)
            for h in range(H):
                qp = ps_q.tile([DK, TOKQ], F32, tag="qp")
                for half in range(2):
                    for fc in range(2):
                        nc.tensor.matmul(
                            qp[:, half * 512:(half + 1) * 512],
                            wq_sb[:, fc * F + h * DK: fc * F + (h + 1) * DK],
                            qt_tiles[fc][:, half * 512:(half + 1) * 512],
                            start=(fc == 0), stop=(fc == 1))
                src = qp.rearrange("p (b q) -> p b q", b=B)
                bias = bq_sb[64 * (h % 2):64 * (h % 2) + 64, h // 2: h // 2 + 1]
                nc.scalar.activation(qv[:, :, :, h], src, AF.Identity, bias=bias)
            nc.gpsimd.dma_start(qq2[64:128, :], qq2[0:DK, :])

        # ===== Phases A+C+D: software-pipelined wavefront emission =====
        # Stages per index: S1(s)=stats, S2(s)=affine+transpose, S3(s)=K/V,
        # S4(qg)=bmat+staging+regroup. Emission is skewed so every engine's
        # queue holds ready work from the next iteration instead of blocking
        # on the tail of the current one.
        NGRP = 32
        with tc.tile_pool(name="poskst", bufs=4) as poskst, \
             tc.tile_pool(name="maskst", bufs=2) as maskst, \
             tc.tile_pool(name="bmstage", bufs=3) as bmstage, \
             tc.tile_pool(name="ps_bm", bufs=2, space="PSUM") as ps_bm:

            pk_tiles = {}
            mk_tiles = {}

            def s_pk(g):
                if not 0 <= g < NGRP:
                    return
                pk = poskst.tile([128, 2 * T], BF16, tag="pk")
                (nc.sync if g % 2 == 0 else nc.gpsimd).dma_start(pk[:], posk_d[g])
                pk_tiles[g] = pk

            def s_mk(m):
                if not 0 <= m < NGRP // 4:
                    return
                mk = maskst.tile([128, T], BF16, tag="mk")
                nc.scalar.dma_start(mk[:], maskp_d[m])
                mk_tiles[m] = mk

            def s4_bmat(g):
                if not 0 <= g < NGRP:
                    return
                pk = pk_tiles.pop(g)
                mk = mk_tiles[g // 4]
                strip = (g % 4) * 32
                qqe = qq2[0:DK].rearrange("p (b h q) -> p b h q", h=H, q=TQ)
                qqo = qq2[64:128].rearrange("p (b h q) -> p b h q", h=H, q=TQ)
                bs = bmstage.tile([128, T], BF16, tag="bs")
                for half in range(2):
                    hs = slice(half * 512, (half + 1) * 512)
                    bp = ps_bm.tile([128, 512], F32, tag="bp")
                    # mask penalty first: writes/sets has_written on the whole
                    # bank; the q-matmuls then accumulate on top.
                    nc.tensor.matmul(bp[:], onehp_sb[strip:strip + 32, :],
                                     mk[strip:strip + 32, hs],
                                     start=True, stop=False,
                                     skip_group_check=True,
                                     tile_position=(strip, 0))
                    for j in range(4):
                        band = 64 * (j % 2)
                        lhs = (qqe if j % 2 == 0 else qqo)[:, :, :, g * 4 + j]
                        nc.tensor.matmul(
                            bp[32 * j:32 * (j + 1), :],
                            lhs,
                            pk[band:band + 64,
                               (j // 2) * T + half * 512:
                               (j // 2) * T + (half + 1) * 512],
                            start=False, stop=(j == 3),
                            skip_group_check=True,
                            tile_position=(band, 32 * j))
                    if half == 0:
                        nc.vector.tensor_copy(bs[:, :512], bp[:])
                    else:
                        nc.scalar.activation(bs[:, 512:], bp[:], AF.Copy)
                if g % 4 == 3:
                    del mk_tiles[g // 4]
                # stage to DRAM (contiguous; regrouped on the way back)
                nc.scalar.dma_start(bounce_d[g], bs[:])

            def s5_fill(m):
                # read back 4 groups: DRAM [(g j), bh, t] -> bmall rows
                # 16m..16m+16 (contiguous partitions)
                src5 = bounce_d[4 * m:4 * (m + 1)].rearrange(
                    "g (j bh) t -> (g j) bh t", j=4)
                dst5 = bmall[16 * m:16 * (m + 1)].rearrange(
                    "p (bh t) -> p bh t", bh=NBH)
                nc.gpsimd.dma_start(dst5, src5)

            with tc.tile_pool(name="xstage", bufs=4) as xstage, \
                 tc.tile_pool(name="xcring", bufs=3) as xcring, \
                 tc.tile_pool(name="stats", bufs=3) as stats, \
                 tc.tile_pool(name="scratch", bufs=2) as scratch, \
                 tc.tile_pool(name="ps_a", bufs=2, space="PSUM") as ps_a, \
                 tc.tile_pool(name="ps_kv", bufs=3, space="PSUM") as ps_kv:

                xt_tiles = {}
                st_out = {}
                xc_tiles = {}

                def s0_load(s):
                    if not 0 <= s < NST:
                        return
                    xt = xstage.tile([128, 4 * F], BF16, tag="xt")
                    nc.sync.dma_start(xt[:], x_d[s])
                    xt_tiles[s] = xt

                def s1_stats(s):
                    if not 0 <= s < NST:
                        return
                    xt = xt_tiles[s]
                    GRP = 4
                    sums = stats.tile([128, GRP], F32, tag="sums")
                    sumsq = stats.tile([128, GRP], F32, tag="sumsq")
                    for c in range(4):
                        nc.vector.reduce_sum(sums[:, c:c + 1],
                                             xt[:, c * F:(c + 1) * F], axis=AXX.X)
                        sq = scratch.tile([128, F], BF16, tag="sq")
                        nc.scalar.activation(sq[:], xt[:, c * F:(c + 1) * F],
                                             AF.Square, accum_out=sumsq[:, c:c + 1])
                    negmu = stats.tile([128, GRP], F32, tag="negmu")
                    nc.vector.tensor_scalar(out=negmu[:], in0=sums[:], scalar1=-1.0 / F,
                                            scalar2=None, op0=ALU.mult)
                    msq = stats.tile([128, GRP], F32, tag="msq")
                    nc.vector.tensor_mul(msq[:], negmu[:], negmu[:])
                    var = stats.tile([128, GRP], F32, tag="var")
                    nc.vector.tensor_scalar(out=var[:], in0=sumsq[:], scalar1=1.0 / F,
                                            scalar2=None, op0=ALU.mult)
                    nc.vector.tensor_tensor(var[:], var[:], msq[:], op=ALU.subtract)
                    sig = stats.tile([128, GRP], F32, tag="sig")
                    nc.scalar.activation(sig[:], var[:], AF.Sqrt, bias=eps_col[:])
                    rsig = stats.tile([128, GRP], F32, tag="rsig")
                    nc.vector.reciprocal(rsig[:], sig[:])
                    cc = stats.tile([128, GRP], F32, tag="cc")
                    nc.vector.tensor_mul(cc[:], negmu[:], rsig[:])
                    st_out[s] = (rsig, cc)

                def s2_affine_t(s):
                    if not 0 <= s < NST:
                        return
                    xt = xt_tiles.pop(s)
                    rsig, cc = st_out.pop(s)
                    xn = scratch.tile([128, 4 * F], BF16, tag="xn")
                    for c in range(4):
                        nc.gpsimd.tensor_scalar(
                            out=xn[:, c * F:(c + 1) * F],
                            in0=xt[:, c * F:(c + 1) * F],
                            scalar1=rsig[:, c:c + 1], scalar2=cc[:, c:c + 1],
                            op0=ALU.mult, op1=ALU.add)
                    # xn^T chunk: [f%128, fc*512 + tok]
                    xc = xcring.tile([128, 2 * 512], BF16, tag="xc")
                    for fc in range(2):
                        tp = ps_a.tile([128, 512], BF16, tag="tp")
                        for c in range(4):
                            nc.tensor.transpose(
                                tp[:, c * 128:(c + 1) * 128],
                                xn[:, c * F + fc * 128: c * F + fc * 128 + 128],
                                id_bf[:])
                        nc.vector.tensor_copy(xc[:, fc * 512:(fc + 1) * 512], tp[:])
                    xc_tiles[s] = xc

                def s3_kv(s):
                    if not 0 <= s < NST:
                        return
                    xc = xc_tiles.pop(s)
                    for ncI in range(2):
                        kp = ps_kv.tile([128, 512], F32, tag="kv")
                        for fc in range(2):
                            nc.tensor.matmul(
                                kp[:],
                                wk_sb[:, fc * F + ncI * 128: fc * F + (ncI + 1) * 128],
                                xc[:, fc * 512:(fc + 1) * 512],
                                start=(fc == 0), stop=(fc == 1))
                        nc.scalar.activation(
                            kt_sb[:, ncI * TOK + s * 512: ncI * TOK + (s + 1) * 512],
                            kp[:], AF.Identity, bias=bk_sb[:, ncI:ncI + 1])
                    for gp in range(2):
                        vp = ps_kv.tile([128, 2 * F], F32, tag="kv")
                        for gi in range(2):
                            g4 = gp * 2 + gi
                            for fc in range(2):
                                nc.tensor.matmul(
                                    vp[:, gi * F:(gi + 1) * F],
                                    xc[:, fc * 512 + g4 * 128: fc * 512 + (g4 + 1) * 128],
                                    wv_sb[:, fc * F:(fc + 1) * F],
                                    start=(fc == 0), stop=(fc == 1 and not bv2_nz))
                            if bv2_nz:
                                nc.tensor.matmul(vp[:, gi * F:(gi + 1) * F],
                                                 ones_bf[:], bv_sb[:],
                                                 start=False, stop=True)
                        g0 = s * 4 + gp * 2
                        nc.vector.tensor_copy(v_sb[:, g0 * F:(g0 + 2) * F], vp[:])

                # Loop A (i=0..9): K/V front-loaded at 2 stages/iter so
                # kt/v complete early; bmat groups 0..15 + fills m=0..3
                # retire alongside.
                s0_load(0)
                s0_load(1)
                s_mk(0)
                for g in range(4):
                    s_pk(g)
                for i in range(NST + 2):
                    s0_load(i + 2)
                    s_pk(2 * i + 4)
                    s_pk(2 * i + 5)
                    if i % 2 == 0:
                        s_mk(i // 2 + 1)
                    s1_stats(i)
                    s2_affine_t(i - 1)
                    s3_kv(i - 2)
                    s4_bmat(2 * (i - 2))
                    s4_bmat(2 * (i - 2) + 1)
                    if i >= 3 and (i - 3) % 2 == 0:
                        s5_fill((i - 3) // 2)

            # ===== Loop B + attention waves =====
            # Wave w covers query rows w*64..w*64+64 (bmall partitions); each
            # step processes a head pair (bhE=2*pi even head, bhO odd head)
            # with row/col-quadrant-packed concurrent matmuls.  Wave 0 is
            # interleaved with the tail of the posk stream; wave 1 follows.
            with tc.tile_pool(name="attw", bufs=2) as attw, \
                 tc.tile_pool(name="ps_ws", bufs=1, space="PSUM") as ps_ws, \
                 tc.tile_pool(name="ps_wt", bufs=2, space="PSUM") as ps_wt, \
                 tc.tile_pool(name="ps_wa", bufs=2, space="PSUM") as ps_wa:

                # per-wave AV accumulators: [q-half rows (E|O), pair*64 + dk]
                avall = [attw.tile([128, 16 * DK], BF16, tag="avall",
                                   name=f"avall{w}") for w in range(2)]

                def wave_stages(w):
                    qlo = w * 64
                    st = {}

                    def w1_scores(pi):
                        if not 0 <= pi < 16:
                            return
                        bhE, bhO = 2 * pi, 2 * pi + 1
                        b, hE = bhE // H, bhE % H
                        hO = hE + 1
                        qaE = qq2[0:DK].rearrange(
                            "p (c q) -> p c q", c=NBH)[:, bhE, qlo:qlo + 64]
                        qaO = qq2[64:128].rearrange(
                            "p (c q) -> p c q", c=NBH)[:, bhO, qlo:qlo + 64]
                        sp2 = ps_ws.tile([128, T], F32, tag="sp2")
                        for half in range(2):
                            hs = slice(half * 512, (half + 1) * 512)
                            ktE = kt_sb[0:64,
                                        (hE // 2) * TOK + b * T + half * 512:
                                        (hE // 2) * TOK + b * T + (half + 1) * 512]
                            ktO = kt_sb[64:128,
                                        (hO // 2) * TOK + b * T + half * 512:
                                        (hO // 2) * TOK + b * T + (half + 1) * 512]
                            nc.tensor.matmul(sp2[:, hs], zrow[:, 0:128],
                                             zrow[:], start=True, stop=False,
                                             skip_group_check=True)
                            nc.tensor.matmul(sp2[0:64, hs], qaE, ktE,
                                             start=False, stop=False,
                                             skip_group_check=True,
                                             tile_position=(0, 0))
                            nc.tensor.matmul(sp2[64:128, hs], qaO, ktO,
                                             start=False, stop=False,
                                             skip_group_check=True,
                                             tile_position=(64, 64))
                            for u in range(4):
                                bhU = bhE if u < 2 else bhO
                                lo = 32 * (u % 2)
                                nc.tensor.matmul(
                                    sp2[32 * u:32 * (u + 1), hs],
                                    id_bf[qlo:qlo + 64,
                                          qlo + lo:qlo + lo + 32],
                                    bmall[qlo:qlo + 64,
                                          bhU * T + half * 512:
                                          bhU * T + (half + 1) * 512],
                                    start=False, stop=True,
                                    skip_group_check=True,
                                    tile_position=(qlo, 32 * u))
                        st[pi] = sp2

                    def w2_soft(pi):
                        if not 0 <= pi < 16:
                            return
                        sp2 = st.pop(pi)
                        attn2 = attw.tile([128, T], BF16, tag="attn2")
                        dn2 = attw.tile([128, 1], F32, tag="dn2")
                        nc.scalar.activation(attn2[:], sp2[:], AF.Exp,
                                             scale=1.0 / np.sqrt(DK),
                                             accum_out=dn2[:])
                        rn2 = attw.tile([128, 1], F32, tag="rn2")
                        nc.vector.reciprocal(rn2[:], dn2[:])
                        st[(pi, 'a')] = (attn2, rn2)

                    def w3_tr(pi):
                        if not 0 <= pi < 16:
                            return
                        attn2, _ = st[(pi, 'a')]
                        a2p = ps_wt.tile([128, T], BF16, tag="a2p")
                        for c in range(8):
                            nc.tensor.transpose(
                                a2p[:, c * 128:(c + 1) * 128],
                                attn2[:, c * 128:(c + 1) * 128], id_bf[:])
                        att2 = attw.tile([128, T], BF16, tag="att2")
                        nc.vector.tensor_copy(att2[:], a2p[:])
                        st[(pi, 't')] = att2

                    def w4_av(pi):
                        if not 0 <= pi < 16:
                            return
                        bhE = 2 * pi
                        b, hE = bhE // H, bhE % H
                        att2 = st.pop((pi, 't'))
                        _, rn2 = st.pop((pi, 'a'))
                        avp2 = ps_wa.tile([128, DK], F32, tag="wa")
                        nc.tensor.matmul(avp2[:], zrow[:, 0:128],
                                         zrow[:, 0:DK], start=True, stop=False,
                                         skip_group_check=True)
                        for c in range(8):
                            vE = v_sb[:, (b * 8 + c) * F + hE * DK:
                                      (b * 8 + c) * F + (hE + 1) * DK]
                            vO = v_sb[:, (b * 8 + c) * F + (hE + 1) * DK:
                                      (b * 8 + c) * F + (hE + 2) * DK]
                            nc.tensor.matmul(
                                avp2[0:64, :], att2[:, c * 128:c * 128 + 64],
                                vE, start=False, stop=(c == 7),
                                skip_group_check=True, tile_position=(0, 0))
                            nc.tensor.matmul(
                                avp2[64:96, :],
                                att2[:, c * 128 + 64:c * 128 + 96],
                                vO, start=False, stop=(c == 7),
                                skip_group_check=True, tile_position=(0, 64))
                            nc.tensor.matmul(
                                avp2[96:128, :],
                                att2[:, c * 128 + 96:c * 128 + 128],
                                vO, start=False, stop=(c == 7),
                                skip_group_check=True, tile_position=(0, 96))
                        nc.vector.tensor_scalar(
                            out=avall[w][:, pi * DK:(pi + 1) * DK],
                            in0=avp2[:], scalar1=rn2[:], scalar2=None,
                            op0=ALU.mult)

                    return w1_scores, w2_soft, w3_tr, w4_av

                w1_0, w2_0, w3_0, w4_0 = wave_stages(0)
                # Loop B (i=10..17): bmat groups 16..31, fills m=4..7, and
                # wave-0 pairs woven into the PE idle time.
                for p in range(17):
                    w1_0(p)
                    w2_0(p - 1)
                    w3_0(p - 1)
                    w4_0(p - 1)

                # wave 1 (posk stream done; runs at pool depth 1)
                w1_1, w2_1, w3_1, w4_1 = wave_stages(1)
                for p in range(17):
                    w1_1(p)
                    w2_1(p - 1)
                    w3_1(p - 1)
                    w4_1(p - 1)

                # ===== gather + output projection per batch =====
                for b in range(B):
                    avt = attw.tile([64, 4 * TQ], F32R, tag="avt", name="avt")
                    for k in range(2):
                        pi = 2 * b + k
                        atp2 = ps_wa.tile([64, 2 * 128], BF16, tag="wa",
                                          name="atp2")
                        for w in range(2):
                            # [128=(qE|qO), 64 dk] -> [64 dk, 128=(qE|qO)]
                            nc.tensor.transpose(
                                atp2[:, w * 128:(w + 1) * 128],
                                avall[w][:, pi * DK:(pi + 1) * DK], id_bf[:])
                        # avt cols for heads 2k (qE) and 2k+1 (qO):
                        # h*TQ + w*64 + q  <-  atp2[:, w*128 + par*64 + q]
                        dst = avt[:, k * 2 * TQ:(k + 1) * 2 * TQ].rearrange(
                            "p (par w q) -> p par w q", par=2, w=2)
                        srcv = atp2.rearrange(
                            "p (w par q) -> p par w q", w=2, par=2)
                        nc.vector.tensor_copy(dst, srcv)
                    op = ps_wa.tile([128, F], F32, tag="wa", name="op")
                    for ci in range(4):
                        nc.tensor.matmul(
                            op[:], avt[:, ci * TQ:(ci + 1) * TQ],
                            wo_sb[:, ci * F:(ci + 1) * F],
                            start=(ci == 0), stop=(ci == 3 and not bo_nz))
                    if bo_nz:
                        nc.tensor.matmul(op[:], ones_r[:], bo_sb[:],
                                         start=False, stop=True)
                    ob = attw.tile([128, F], F32, tag="ob")
                    nc.vector.tensor_copy(ob[:], op[:])
                    nc.gpsimd.dma_start(out_d[b], ob[:])

    nc._dbg_names = {
        "kt": kt_sb.tensor.name, "v": v_sb.tensor.name,
        "qq": qq2.tensor.name, "bmall": bmall.tensor.name,
    }
    nc.compile()
    return nc


def make_core_inputs(inputs, cfg=CFG):
    """Host-side sharding/layout. Returns (per_core_maps, bias_flags)."""
    x = np.asarray(inputs["x"], np.float32)
    q_in = np.asarray(inputs["q_in"], np.float32)
    pos_k = np.asarray(inputs["pos_k"], np.float32)
    mask = np.asarray(inputs["mask"])
    ln_g = np.asarray(inputs["ln_g"], np.float32)
    ln_b = np.asarray(inputs["ln_b"], np.float32)
    Wq, bq = np.asarray(inputs["Wq"], np.float32), np.asarray(inputs["bq"], np.float32)
    Wk, bk = np.asarray(inputs["Wk"], np.float32), np.asarray(inputs["bk"], np.float32)
    Wv, bv = np.asarray(inputs["Wv"], np.float32), np.asarray(inputs["bv"], np.float32)
    Wo, bo = np.asarray(inputs["Wo"], np.float32), np.asarray(inputs["bo"], np.float32)

    bf = ml_dtypes.bfloat16
    Wk2 = ln_g[:, None] * Wk
    bk2 = ln_b @ Wk + bk
    Wv2 = ln_g[:, None] * Wv
    bv2 = ln_b @ Wv + bv

    xp = np.ascontiguousarray(
        x.reshape(TOK // 512, 4, 128, F).transpose(0, 2, 1, 3).reshape(
            TOK // 512, 128, 4 * F)).astype(bf)
    shared = {
        "xp": xp,
        "wq": Wq.astype(bf),
        "wk2": Wk2.astype(bf),
        "wv2": Wv2.astype(bf),
        "wo": Wo.astype(np.float32),
        "bq_cols": np.ascontiguousarray(bq.reshape(2, 128).T).astype(np.float32),
        "bk2_cols": np.ascontiguousarray(bk2.reshape(2, 128).T).astype(np.float32),
        "bv2_row": bv2.reshape(1, F).astype(bf),
        "bo_row": bo.reshape(1, F).astype(np.float32),
        "ones_row": np.ones((1, 128), np.float32),
    }
    # mask-matmul stationary: rows s*32+j*8+b (same for each strip s),
    # cols j*32 + b*H + h = -PEN
    op = np.zeros((4, 4, B, 128), np.float32)
    for j in range(4):
        for bb in range(B):
            op[:, j, bb, j * 32 + bb * H:j * 32 + (bb + 1) * H] = -PEN
    shared["onehp"] = np.ascontiguousarray(op.reshape(128, 128)).astype(bf)


    per_core = []
    for c in range(NCORES):
        qs = slice(c * TQ, (c + 1) * TQ)
        # posk: [g, band*64+d, pair*T+t]; band = local q parity, pair = q//2%2
        pkt = pos_k[qs].transpose(0, 2, 1).astype(bf)      # [q, d, t]
        A = pkt.reshape(TQ // 4, 4, DK, T)                 # [g, j, d, t]
        top = np.concatenate([A[:, 0], A[:, 2]], axis=2)   # [g, 64, 2T]
        bot = np.concatenate([A[:, 1], A[:, 3]], axis=2)
        pa = np.ascontiguousarray(np.concatenate([top, bot], axis=1))
        # mask: [m, s*32+j*8+b, t] = 1-mask[b, 16m+4s+j, t]
        mm = (1.0 - mask[:, qs, :].astype(np.float32)).transpose(1, 0, 2)
        mp = np.ascontiguousarray(
            mm.reshape(TQ // 16, 4, 4, B, T).reshape(TQ // 16, 128, T)
        ).astype(bf)
        qt = np.ascontiguousarray(q_in[:, qs, :].reshape(TOKQ, F).T).astype(bf)
        m = dict(shared)
        m["posk_p"] = pa
        m["mask_p"] = mp
        m["q_t"] = qt
        per_core.append(m)
    flags = dict(bv2_nz=bool(np.any(bv2)), bo_nz=bool(np.any(bo)))
    return per_core, flags


_PROGRAM_CACHE = {}


def kernel(**inputs):
    per_core, flags = make_core_inputs(inputs, CFG)
    key = (tuple(sorted(CFG.items())), tuple(sorted(flags.items())))
    if key not in _PROGRAM_CACHE:
        _PROGRAM_CACHE[key] = build_program(CFG, **flags)
    nc = _PROGRAM_CACHE[key]
    res = run_bass_kernel_spmd(nc, per_core, core_ids=list(range(NCORES)))
    outs = [res.results[c]["out"] for c in range(NCORES)]
    return np.concatenate(outs, axis=1).astype(np.float32)



# revision 45
# speedup vs baseline: 1.0403x; 1.0403x over previous
"""Cross multi-headed attention with relative position bias, sharded over 8 trn2 cores.

Sharding: query positions (T1=1024) split 128/core. pos_k (the 256MB tensor) is
indexed by query position, so each core streams a disjoint 1/8 slice exactly
once. K/V are computed from the full x on every core (cheap); outputs are
disjoint q-slices gathered on host.

Layout tricks:
  - pos_k is host-transposed/packed to (qgroup, dk+B, 2*T2) so it streams as
    the matmul moving operand (contraction dim dk on partitions) in 294KB DMAs,
    self-paced on the SP engine (a dedicated queue that blocks on pool bufs).
  - The mask penalty is folded into the Bmat matmul as 8 extra contraction
    rows (one-hot per batch x -1e4 penalty): masking costs zero vector ops and
    masked attn weights are exactly 0 after exp underflow.
  - Bmat comes out of its matmul in per-q tiles (32 bh-rows x T2); an
    SBUF->SBUF permute DMA regroups it into a resident 8MB SBUF buffer of
    per-(b,h) tiles (128q x T2) - no DRAM roundtrip - which an identity-matmul
    accumulates straight onto the QK^T scores in PSUM.
  - xn^T lives in a small ring (consumed per-chunk by the K/V projections),
    freeing the SBUF needed for the resident Bmat buffer.
  - LayerNorm gain/bias are folded into the projection weights on host;
    stats come from one bn_stats/bn_aggr pass; mean/rstd applied in natural
    layout (GPSIMD), then xn is PE-transposed for the projections.
  - Softmax denominators come from the exp activation's accum_out for free;
    no max-subtraction needed since scores are bounded (~N(0,2) after scale).
"""

import numpy as np
import ml_dtypes
from contextlib import ExitStack

import concourse.bass as bass
import concourse.tile as tile
import concourse.mybir as mybir
from concourse import bacc
from concourse.bass_utils import run_bass_kernel_spmd
from concourse.masks import make_identity

F32 = mybir.dt.float32
F32R = mybir.dt.float32r
BF16 = mybir.dt.bfloat16
FP8 = mybir.dt.float8e3

B, T, F, H, DK = 8, 1024, 256, 4, 64
NCORES = 8
TQ = T // NCORES          # query rows per core (128)
TOK = B * T               # tokens for K/V (8192)
TOKQ = B * TQ             # query tokens per core (1024)
NBH = B * H               # 32 (b,h) pairs
KAUG = DK + B             # bmat contraction rows: 64 qk + 8 mask-penalty
EPS = 1e-5
PEN = 1e4                 # mask penalty (score -= PEN/8 => exp underflows to 0)

CFG = dict()

AF = mybir.ActivationFunctionType
ALU = mybir.AluOpType
AXX = mybir.AxisListType


def build_program(cfg=CFG, bv2_nz=False, bo_nz=False):
    nc = bacc.Bacc("TRN2", target_bir_lowering=False, debug=False)

    # ---- DRAM I/O ----
    # x packed as 16 super-tiles: [st, tok%128, (tok//128 % 4)*F + f]
    x_d = nc.dram_tensor("xp", [TOK // 512, 128, 4 * F], BF16, kind="ExternalInput")
    qt_d = nc.dram_tensor("q_t", [F, TOKQ], BF16, kind="ExternalInput")
    # pos_k packed 4 q per tile on all 128 partitions (4KB lines):
    # rows 0:64 = d-rows of even local q, 64:128 = odd; cols 0:T = q-pair A,
    # T:2T = q-pair B.  [g, band*64+d, pair*T + t]
    posk_d = nc.dram_tensor("posk_p", [TQ // 4, 128, 2 * T], BF16,
                            kind="ExternalInput")
    # mask penalty operand: tile m covers groups 4m..4m+3 (16 q); row
    # s*32 + j*8 + b = (1-mask[b, q=16m+4s+j, t])
    maskp_d = nc.dram_tensor("mask_p", [TQ // 16, 128, T], BF16,
                             kind="ExternalInput")
    # stationary for the mask matmul: [s*32+j*8+b, j*32+b*H+h] = -PEN
    onehp_d = nc.dram_tensor("onehp", [128, 128], BF16, kind="ExternalInput")
    # DRAM bounce for the Bmat regroup: written per-group (contiguous, all 16
    # DMA ports), read back per-8-group batch into contiguous 32-partition
    # blocks of bmall (8 ports) - avoids the 4-partition SBUF->SBUF scatter.
    bounce_d = nc.dram_tensor("bm_bounce", [TQ // 4, 128, T], BF16,
                              kind="Internal")
    wq_d = nc.dram_tensor("wq", [F, F], BF16, kind="ExternalInput")
    wk_d = nc.dram_tensor("wk2", [F, F], BF16, kind="ExternalInput")
    wv_d = nc.dram_tensor("wv2", [F, F], BF16, kind="ExternalInput")
    wo_d = nc.dram_tensor("wo", [F, F], F32R, kind="ExternalInput")
    bq_d = nc.dram_tensor("bq_cols", [128, 2], F32, kind="ExternalInput")
    bk_d = nc.dram_tensor("bk2_cols", [128, 2], F32, kind="ExternalInput")
    bv_d = nc.dram_tensor("bv2_row", [1, F], BF16, kind="ExternalInput")
    bo_d = nc.dram_tensor("bo_row", [1, F], F32R, kind="ExternalInput")
    ones_d = nc.dram_tensor("ones_row", [1, 128], F32R, kind="ExternalInput")
    out_d = nc.dram_tensor("out", [B, TQ, F], F32, kind="ExternalOutput")

    with tile.TileContext(nc) as tc, ExitStack() as ctx:
        consts = ctx.enter_context(tc.tile_pool(name="consts", bufs=1))
        persist = ctx.enter_context(tc.tile_pool(name="persist", bufs=1))

        # ---- constants (loaded on sync before the posk stream starts) ----
        id_bf = consts.tile([128, 128], BF16)
        make_identity(nc, id_bf)
        id_f32 = consts.tile([128, 128], F32)
        make_identity(nc, id_f32)
        wq_sb = consts.tile([128, 2 * F], BF16)   # [f%128, (f//128)*F + n]
        wk_sb = consts.tile([128, 2 * F], BF16)
        wv_sb = consts.tile([128, 2 * F], BF16)
        wo_sb = consts.tile([64, 4 * F], F32R)    # [hd%64, (hd//64)*F + n]
        onehp_sb = consts.tile([128, 128], BF16)
        nc.sync.dma_start(onehp_sb[:], onehp_d[:])
        zrow = consts.tile([1, 512], BF16)
        nc.gpsimd.memset(zrow[:], 0.0)
        for w_sb, w_d in ((wq_sb, wq_d), (wk_sb, wk_d), (wv_sb, wv_d)):
            for fc in range(2):
                nc.sync.dma_start(w_sb[:, fc * F:(fc + 1) * F],
                                  w_d[fc * 128:(fc + 1) * 128, :])
        for ci in range(4):
            nc.sync.dma_start(wo_sb[:, ci * F:(ci + 1) * F],
                              wo_d[ci * 64:(ci + 1) * 64, :])
        bq_sb = consts.tile([128, 2], F32)
        nc.sync.dma_start(bq_sb[:], bq_d[:])
        bk_sb = consts.tile([128, 2], F32)
        nc.sync.dma_start(bk_sb[:], bk_d[:])
        if bv2_nz:
            bv_sb = consts.tile([1, F], BF16)
            nc.sync.dma_start(bv_sb[:], bv_d[:])
            ones_bf = consts.tile([1, 128], BF16)
            nc.gpsimd.memset(ones_bf[:], 1.0)
        if bo_nz:
            bo_sb = consts.tile([1, F], F32R)
            nc.sync.dma_start(bo_sb[:], bo_d[:])
            ones_r = consts.tile([1, 128], F32R)
            nc.sync.dma_start(ones_r[:], ones_d[:])
        eps_col = consts.tile([128, 1], F32)
        nc.gpsimd.memset(eps_col[:], EPS)

        # ---- persistent activations ----
        kt_sb = persist.tile([128, 2 * TOK], BF16)     # K^T: [n%128, (n//128)*TOK+tok]
        v_sb = persist.tile([128, (TOK // 128) * F], BF16)  # V: [tok%128,(tok//128)*F+hd]
        # rows 0:64 = Q^T [d, b*H*TQ + h*TQ + q]; rows 64:128 = copy of 0:64
        qq2 = persist.tile([128, TQ * NBH], BF16)
        # resident regrouped Bmat: [q, bh*T + t]  (8MB, 64KB/partition)
        bmall = persist.tile([128, NBH * T], BF16)

        NST = TOK // 512  # 16 super tiles

        # ================= Phase B: Q^T projection -> qq_sb =================
        with tc.tile_pool(name="qstage", bufs=1) as qstage, \
             tc.tile_pool(name="ps_q", bufs=2, space="PSUM") as ps_q:
            qt_tiles = []
            for fc in range(2):
                qt = qstage.tile([128, TOKQ], BF16, tag=f"qt{fc}")
                nc.scalar.dma_start(qt[:], qt_d[fc * 128:(fc + 1) * 128, :])
                qt_tiles.append(qt)
            qv = qq2[0:DK].rearrange("p (b h q) -> p b q h", h=H, q=TQ)
            for h in range(H):
                qp = ps_q.tile([DK, TOKQ], F32, tag="qp")
                for half in range(2):
                    for fc in range(2):
                        nc.tensor.matmul(
                            qp[:, half * 512:(half + 1) * 512],
                            wq_sb[:, fc * F + h * DK: fc * F + (h + 1) * DK],
                            qt_tiles[fc][:, half * 512:(half + 1) * 512],
                            start=(fc == 0), stop=(fc == 1))
                src = qp.rearrange("p (b q) -> p b q", b=B)
                bias = bq_sb[64 * (h % 2):64 * (h % 2) + 64, h // 2: h // 2 + 1]
                nc.scalar.activation(qv[:, :, :, h], src, AF.Identity, bias=bias)
            nc.gpsimd.dma_start(qq2[64:128, :], qq2[0:DK, :])

        # ===== Phases A+C+D: software-pipelined wavefront emission =====
        # Stages per index: S1(s)=stats, S2(s)=affine+transpose, S3(s)=K/V,
        # S4(qg)=bmat+staging+regroup. Emission is skewed so every engine's
        # queue holds ready work from the next iteration instead of blocking
        # on the tail of the current one.
        NGRP = 32
        with tc.tile_pool(name="poskst", bufs=4) as poskst, \
             tc.tile_pool(name="maskst", bufs=2) as maskst, \
             tc.tile_pool(name="bmstage", bufs=3) as bmstage, \
             tc.tile_pool(name="ps_bm", bufs=2, space="PSUM") as ps_bm:

            pk_tiles = {}
            mk_tiles = {}

            def s_pk(g):
                if not 0 <= g < NGRP:
                    return
                pk = poskst.tile([128, 2 * T], BF16, tag="pk")
                (nc.sync if g % 2 == 0 else nc.gpsimd).dma_start(pk[:], posk_d[g])
                pk_tiles[g] = pk

            def s_mk(m):
                if not 0 <= m < NGRP // 4:
                    return
                mk = maskst.tile([128, T], BF16, tag="mk")
                nc.scalar.dma_start(mk[:], maskp_d[m])
                mk_tiles[m] = mk

            def s4_bmat(g):
                if not 0 <= g < NGRP:
                    return
                pk = pk_tiles.pop(g)
                mk = mk_tiles[g // 4]
                strip = (g % 4) * 32
                qqe = qq2[0:DK].rearrange("p (b h q) -> p b h q", h=H, q=TQ)
                qqo = qq2[64:128].rearrange("p (b h q) -> p b h q", h=H, q=TQ)
                bs = bmstage.tile([128, T], BF16, tag="bs")
                for half in range(2):
                    hs = slice(half * 512, (half + 1) * 512)
                    bp = ps_bm.tile([128, 512], F32, tag="bp")
                    # mask penalty first: writes/sets has_written on the whole
                    # bank; the q-matmuls then accumulate on top.
                    nc.tensor.matmul(bp[:], onehp_sb[strip:strip + 32, :],
                                     mk[strip:strip + 32, hs],
                                     start=True, stop=False,
                                     skip_group_check=True,
                                     tile_position=(strip, 0))
                    for j in range(4):
                        band = 64 * (j % 2)
                        lhs = (qqe if j % 2 == 0 else qqo)[:, :, :, g * 4 + j]
                        nc.tensor.matmul(
                            bp[32 * j:32 * (j + 1), :],
                            lhs,
                            pk[band:band + 64,
                               (j // 2) * T + half * 512:
                               (j // 2) * T + (half + 1) * 512],
                            start=False, stop=(j == 3),
                            skip_group_check=True,
                            tile_position=(band, 32 * j))
                    if half == 0:
                        nc.vector.tensor_copy(bs[:, :512], bp[:])
                    else:
                        nc.scalar.activation(bs[:, 512:], bp[:], AF.Copy)
                if g % 4 == 3:
                    del mk_tiles[g // 4]
                # stage to DRAM (contiguous; regrouped on the way back)
                nc.scalar.dma_start(bounce_d[g], bs[:])

            def s5_fill(m):
                # read back 4 groups: DRAM [(g j), bh, t] -> bmall rows
                # 16m..16m+16 (contiguous partitions)
                src5 = bounce_d[4 * m:4 * (m + 1)].rearrange(
                    "g (j bh) t -> (g j) bh t", j=4)
                dst5 = bmall[16 * m:16 * (m + 1)].rearrange(
                    "p (bh t) -> p bh t", bh=NBH)
                nc.sync.dma_start(dst5, src5)

            with tc.tile_pool(name="xstage", bufs=4) as xstage, \
                 tc.tile_pool(name="xcring", bufs=3) as xcring, \
                 tc.tile_pool(name="stats", bufs=3) as stats, \
                 tc.tile_pool(name="scratch", bufs=2) as scratch, \
                 tc.tile_pool(name="ps_a", bufs=2, space="PSUM") as ps_a, \
                 tc.tile_pool(name="ps_kv", bufs=3, space="PSUM") as ps_kv:

                xt_tiles = {}
                st_out = {}
                xc_tiles = {}

                def s0_load(s):
                    if not 0 <= s < NST:
                        return
                    xt = xstage.tile([128, 4 * F], BF16, tag="xt")
                    nc.sync.dma_start(xt[:], x_d[s])
                    xt_tiles[s] = xt

                def s1_stats(s):
                    if not 0 <= s < NST:
                        return
                    xt = xt_tiles[s]
                    GRP = 4
                    sums = stats.tile([128, GRP], F32, tag="sums")
                    sumsq = stats.tile([128, GRP], F32, tag="sumsq")
                    for c in range(4):
                        nc.vector.reduce_sum(sums[:, c:c + 1],
                                             xt[:, c * F:(c + 1) * F], axis=AXX.X)
                        sq = scratch.tile([128, F], BF16, tag="sq")
                        nc.scalar.activation(sq[:], xt[:, c * F:(c + 1) * F],
                                             AF.Square, accum_out=sumsq[:, c:c + 1])
                    negmu = stats.tile([128, GRP], F32, tag="negmu")
                    nc.vector.tensor_scalar(out=negmu[:], in0=sums[:], scalar1=-1.0 / F,
                                            scalar2=None, op0=ALU.mult)
                    msq = stats.tile([128, GRP], F32, tag="msq")
                    nc.vector.tensor_mul(msq[:], negmu[:], negmu[:])
                    var = stats.tile([128, GRP], F32, tag="var")
                    nc.vector.tensor_scalar(out=var[:], in0=sumsq[:], scalar1=1.0 / F,
                                            scalar2=None, op0=ALU.mult)
                    nc.vector.tensor_tensor(var[:], var[:], msq[:], op=ALU.subtract)
                    sig = stats.tile([128, GRP], F32, tag="sig")
                    nc.scalar.activation(sig[:], var[:], AF.Sqrt, bias=eps_col[:])
                    rsig = stats.tile([128, GRP], F32, tag="rsig")
                    nc.vector.reciprocal(rsig[:], sig[:])
                    cc = stats.tile([128, GRP], F32, tag="cc")
                    nc.vector.tensor_mul(cc[:], negmu[:], rsig[:])
                    st_out[s] = (rsig, cc)

                def s2_affine_t(s):
                    if not 0 <= s < NST:
                        return
                    xt = xt_tiles.pop(s)
                    rsig, cc = st_out.pop(s)
                    xn = scratch.tile([128, 4 * F], BF16, tag="xn")
                    for c in range(4):
                        nc.gpsimd.tensor_scalar(
                            out=xn[:, c * F:(c + 1) * F],
                            in0=xt[:, c * F:(c + 1) * F],
                            scalar1=rsig[:, c:c + 1], scalar2=cc[:, c:c + 1],
                            op0=ALU.mult, op1=ALU.add)
                    # xn^T chunk: [f%128, fc*512 + tok]
                    xc = xcring.tile([128, 2 * 512], BF16, tag="xc")
                    for fc in range(2):
                        tp = ps_a.tile([128, 512], BF16, tag="tp")
                        for c in range(4):
                            nc.tensor.transpose(
                                tp[:, c * 128:(c + 1) * 128],
                                xn[:, c * F + fc * 128: c * F + fc * 128 + 128],
                                id_bf[:])
                        nc.vector.tensor_copy(xc[:, fc * 512:(fc + 1) * 512], tp[:])
                    xc_tiles[s] = xc

                def s3_kv(s):
                    if not 0 <= s < NST:
                        return
                    xc = xc_tiles.pop(s)
                    for ncI in range(2):
                        kp = ps_kv.tile([128, 512], F32, tag="kv")
                        for fc in range(2):
                            nc.tensor.matmul(
                                kp[:],
                                wk_sb[:, fc * F + ncI * 128: fc * F + (ncI + 1) * 128],
                                xc[:, fc * 512:(fc + 1) * 512],
                                start=(fc == 0), stop=(fc == 1))
                        nc.scalar.activation(
                            kt_sb[:, ncI * TOK + s * 512: ncI * TOK + (s + 1) * 512],
                            kp[:], AF.Identity, bias=bk_sb[:, ncI:ncI + 1])
                    for gp in range(2):
                        vp = ps_kv.tile([128, 2 * F], F32, tag="kv")
                        for gi in range(2):
                            g4 = gp * 2 + gi
                            for fc in range(2):
                                nc.tensor.matmul(
                                    vp[:, gi * F:(gi + 1) * F],
                                    xc[:, fc * 512 + g4 * 128: fc * 512 + (g4 + 1) * 128],
                                    wv_sb[:, fc * F:(fc + 1) * F],
                                    start=(fc == 0), stop=(fc == 1 and not bv2_nz))
                            if bv2_nz:
                                nc.tensor.matmul(vp[:, gi * F:(gi + 1) * F],
                                                 ones_bf[:], bv_sb[:],
                                                 start=False, stop=True)
                        g0 = s * 4 + gp * 2
                        nc.vector.tensor_copy(v_sb[:, g0 * F:(g0 + 2) * F], vp[:])

                # Loop A (i=0..9): K/V front-loaded at 2 stages/iter so
                # kt/v complete early; bmat groups 0..15 + fills m=0..3
                # retire alongside.
                s0_load(0)
                s0_load(1)
                s_mk(0)
                for g in range(4):
                    s_pk(g)
                for i in range(10):
                    s1_stats(2 * i)
                    s1_stats(2 * i + 1)
                    s2_affine_t(2 * i - 2)
                    s2_affine_t(2 * i - 1)
                    s0_load(2 * i + 2)
                    s0_load(2 * i + 3)
                    s_pk(2 * i + 4)
                    s_pk(2 * i + 5)
                    if i % 2 == 0:
                        s_mk(i // 2 + 1)
                    s3_kv(2 * i - 4)
                    s3_kv(2 * i - 3)
                    s4_bmat(2 * (i - 2))
                    s4_bmat(2 * (i - 2) + 1)
                    if i >= 3 and (i - 3) % 2 == 0:
                        s5_fill((i - 3) // 2)

            # ===== Loop B + attention waves =====
            # Wave w covers query rows w*64..w*64+64 (bmall partitions); each
            # step processes a head pair (bhE=2*pi even head, bhO odd head)
            # with row/col-quadrant-packed concurrent matmuls.  Wave 0 is
            # interleaved with the tail of the posk stream; wave 1 follows.
            with tc.tile_pool(name="attw", bufs=2) as attw, \
                 tc.tile_pool(name="ps_ws", bufs=1, space="PSUM") as ps_ws, \
                 tc.tile_pool(name="ps_wt", bufs=2, space="PSUM") as ps_wt, \
                 tc.tile_pool(name="ps_wa", bufs=2, space="PSUM") as ps_wa:

                # per-wave AV accumulators: [q-half rows (E|O), pair*64 + dk]
                avall = [attw.tile([128, 16 * DK], BF16, tag="avall",
                                   name=f"avall{w}") for w in range(2)]

                def wave_stages(w):
                    qlo = w * 64
                    st = {}

                    def w1_scores(pi):
                        if not 0 <= pi < 16:
                            return
                        bhE, bhO = 2 * pi, 2 * pi + 1
                        b, hE = bhE // H, bhE % H
                        hO = hE + 1
                        qaE = qq2[0:DK].rearrange(
                            "p (c q) -> p c q", c=NBH)[:, bhE, qlo:qlo + 64]
                        qaO = qq2[64:128].rearrange(
                            "p (c q) -> p c q", c=NBH)[:, bhO, qlo:qlo + 64]
                        sp2 = ps_ws.tile([128, T], F32, tag="sp2")
                        for half in range(2):
                            hs = slice(half * 512, (half + 1) * 512)
                            ktE = kt_sb[0:64,
                                        (hE // 2) * TOK + b * T + half * 512:
                                        (hE // 2) * TOK + b * T + (half + 1) * 512]
                            ktO = kt_sb[64:128,
                                        (hO // 2) * TOK + b * T + half * 512:
                                        (hO // 2) * TOK + b * T + (half + 1) * 512]
                            nc.tensor.matmul(sp2[:, hs], zrow[:, 0:128],
                                             zrow[:], start=True, stop=False,
                                             skip_group_check=True)
                            nc.tensor.matmul(sp2[0:64, hs], qaE, ktE,
                                             start=False, stop=False,
                                             skip_group_check=True,
                                             tile_position=(0, 0))
                            nc.tensor.matmul(sp2[64:128, hs], qaO, ktO,
                                             start=False, stop=False,
                                             skip_group_check=True,
                                             tile_position=(64, 64))
                            for u in range(4):
                                bhU = bhE if u < 2 else bhO
                                lo = 32 * (u % 2)
                                nc.tensor.matmul(
                                    sp2[32 * u:32 * (u + 1), hs],
                                    id_bf[qlo:qlo + 64,
                                          qlo + lo:qlo + lo + 32],
                                    bmall[qlo:qlo + 64,
                                          bhU * T + half * 512:
                                          bhU * T + (half + 1) * 512],
                                    start=False, stop=True,
                                    skip_group_check=True,
                                    tile_position=(qlo, 32 * u))
                        st[pi] = sp2

                    def w2_soft(pi):
                        if not 0 <= pi < 16:
                            return
                        sp2 = st.pop(pi)
                        attn2 = attw.tile([128, T], BF16, tag="attn2")
                        dn2 = attw.tile([128, 1], F32, tag="dn2")
                        nc.scalar.activation(attn2[:], sp2[:], AF.Exp,
                                             scale=1.0 / np.sqrt(DK),
                                             accum_out=dn2[:])
                        rn2 = attw.tile([128, 1], F32, tag="rn2")
                        nc.vector.reciprocal(rn2[:], dn2[:])
                        st[(pi, 'a')] = (attn2, rn2)

                    def w3_tr(pi):
                        if not 0 <= pi < 16:
                            return
                        attn2, _ = st[(pi, 'a')]
                        a2p = ps_wt.tile([128, T], BF16, tag="a2p")
                        for c in range(8):
                            nc.tensor.transpose(
                                a2p[:, c * 128:(c + 1) * 128],
                                attn2[:, c * 128:(c + 1) * 128], id_bf[:])
                        att2 = attw.tile([128, T], BF16, tag="att2")
                        nc.vector.tensor_copy(att2[:], a2p[:])
                        st[(pi, 't')] = att2

                    def w4_av(pi):
                        if not 0 <= pi < 16:
                            return
                        bhE = 2 * pi
                        b, hE = bhE // H, bhE % H
                        att2 = st.pop((pi, 't'))
                        _, rn2 = st.pop((pi, 'a'))
                        avp2 = ps_wa.tile([128, DK], F32, tag="wa")
                        nc.tensor.matmul(avp2[:], zrow[:, 0:128],
                                         zrow[:, 0:DK], start=True, stop=False,
                                         skip_group_check=True)
                        for c in range(8):
                            vE = v_sb[:, (b * 8 + c) * F + hE * DK:
                                      (b * 8 + c) * F + (hE + 1) * DK]
                            vO = v_sb[:, (b * 8 + c) * F + (hE + 1) * DK:
                                      (b * 8 + c) * F + (hE + 2) * DK]
                            nc.tensor.matmul(
                                avp2[0:64, :], att2[:, c * 128:c * 128 + 64],
                                vE, start=False, stop=(c == 7),
                                skip_group_check=True, tile_position=(0, 0))
                            nc.tensor.matmul(
                                avp2[64:96, :],
                                att2[:, c * 128 + 64:c * 128 + 96],
                                vO, start=False, stop=(c == 7),
                                skip_group_check=True, tile_position=(0, 64))
                            nc.tensor.matmul(
                                avp2[96:128, :],
                                att2[:, c * 128 + 96:c * 128 + 128],
                                vO, start=False, stop=(c == 7),
                                skip_group_check=True, tile_position=(0, 96))
                        nc.vector.tensor_scalar(
                            out=avall[w][:, pi * DK:(pi + 1) * DK],
                            in0=avp2[:], scalar1=rn2[:], scalar2=None,
                            op0=ALU.mult)

                    return w1_scores, w2_soft, w3_tr, w4_av

                w1_0, w2_0, w3_0, w4_0 = wave_stages(0)
                # Loop B (i=10..17): bmat groups 16..31, fills m=4..7, and
                # wave-0 pairs woven into the PE idle time.
                for i in range(10, 18):
                    s_pk(2 * i + 4)
                    s_pk(2 * i + 5)
                    if i % 2 == 0:
                        s_mk(i // 2 + 1)
                    s4_bmat(2 * (i - 2))
                    s4_bmat(2 * (i - 2) + 1)
                    if (i - 3) % 2 == 0:
                        s5_fill((i - 3) // 2)
                    for p in (2 * (i - 10), 2 * (i - 10) + 1):
                        w2_0(p - 1)
                        w3_0(p - 1)
                        w4_0(p - 1)
                        w1_0(p)
                w2_0(15)
                w3_0(15)
                w4_0(15)

                # wave 1 (posk stream done; runs at pool depth 1)
                w1_1, w2_1, w3_1, w4_1 = wave_stages(1)
                for p in range(17):
                    w1_1(p)
                    w2_1(p - 1)
                    w3_1(p - 1)
                    w4_1(p - 1)

                # ===== gather + output projection per batch =====
                for b in range(B):
                    avt = attw.tile([64, 4 * TQ], F32R, tag="avt", name="avt")
                    for k in range(2):
                        pi = 2 * b + k
                        atp2 = ps_wa.tile([64, 2 * 128], BF16, tag="wa",
                                          name="atp2")
                        for w in range(2):
                            # [128=(qE|qO), 64 dk] -> [64 dk, 128=(qE|qO)]
                            nc.tensor.transpose(
                                atp2[:, w * 128:(w + 1) * 128],
                                avall[w][:, pi * DK:(pi + 1) * DK], id_bf[:])
                        # avt cols for heads 2k (qE) and 2k+1 (qO):
                        # h*TQ + w*64 + q  <-  atp2[:, w*128 + par*64 + q]
                        dst = avt[:, k * 2 * TQ:(k + 1) * 2 * TQ].rearrange(
                            "p (par w q) -> p par w q", par=2, w=2)
                        srcv = atp2.rearrange(
                            "p (w par q) -> p par w q", w=2, par=2)
                        nc.vector.tensor_copy(dst, srcv)
                    op = ps_wa.tile([128, F], F32, tag="wa", name="op")
                    for ci in range(4):
                        nc.tensor.matmul(
                            op[:], avt[:, ci * TQ:(ci + 1) * TQ],
                            wo_sb[:, ci * F:(ci + 1) * F],
                            start=(ci == 0), stop=(ci == 3 and not bo_nz))
                    if bo_nz:
                        nc.tensor.matmul(op[:], ones_r[:], bo_sb[:],
                                         start=False, stop=True)
                    ob = attw.tile([128, F], F32, tag="ob")
                    nc.vector.tensor_copy(ob[:], op[:])
                    nc.gpsimd.dma_start(out_d[b], ob[:])

    nc._dbg_names = {
        "kt": kt_sb.tensor.name, "v": v_sb.tensor.name,
        "qq": qq2.tensor.name, "bmall": bmall.tensor.name,
    }
    nc.compile()
    return nc


def make_core_inputs(inputs, cfg=CFG):
    """Host-side sharding/layout. Returns (per_core_maps, bias_flags)."""
    x = np.asarray(inputs["x"], np.float32)
    q_in = np.asarray(inputs["q_in"], np.float32)
    pos_k = np.asarray(inputs["pos_k"], np.float32)
    mask = np.asarray(inputs["mask"])
    ln_g = np.asarray(inputs["ln_g"], np.float32)
    ln_b = np.asarray(inputs["ln_b"], np.float32)
    Wq, bq = np.asarray(inputs["Wq"], np.float32), np.asarray(inputs["bq"], np.float32)
    Wk, bk = np.asarray(inputs["Wk"], np.float32), np.asarray(inputs["bk"], np.float32)
    Wv, bv = np.asarray(inputs["Wv"], np.float32), np.asarray(inputs["bv"], np.float32)
    Wo, bo = np.asarray(inputs["Wo"], np.float32), np.asarray(inputs["bo"], np.float32)

    bf = ml_dtypes.bfloat16
    Wk2 = ln_g[:, None] * Wk
    bk2 = ln_b @ Wk + bk
    Wv2 = ln_g[:, None] * Wv
    bv2 = ln_b @ Wv + bv

    xp = np.ascontiguousarray(
        x.reshape(TOK // 512, 4, 128, F).transpose(0, 2, 1, 3).reshape(
            TOK // 512, 128, 4 * F)).astype(bf)
    shared = {
        "xp": xp,
        "wq": Wq.astype(bf),
        "wk2": Wk2.astype(bf),
        "wv2": Wv2.astype(bf),
        "wo": Wo.astype(np.float32),
        "bq_cols": np.ascontiguousarray(bq.reshape(2, 128).T).astype(np.float32),
        "bk2_cols": np.ascontiguousarray(bk2.reshape(2, 128).T).astype(np.float32),
        "bv2_row": bv2.reshape(1, F).astype(bf),
        "bo_row": bo.reshape(1, F).astype(np.float32),
        "ones_row": np.ones((1, 128), np.float32),
    }
    # mask-matmul stationary: rows s*32+j*8+b (same for each strip s),
    # cols j*32 + b*H + h = -PEN
    op = np.zeros((4, 4, B, 128), np.float32)
    for j in range(4):
        for bb in range(B):
            op[:, j, bb, j * 32 + bb * H:j * 32 + (bb + 1) * H] = -PEN
    shared["onehp"] = np.ascontiguousarray(op.reshape(128, 128)).astype(bf)


    per_core = []
    for c in range(NCORES):
        qs = slice(c * TQ, (c + 1) * TQ)
        # posk: [g, band*64+d, pair*T+t]; band = local q parity, pair = q//2%2
        pkt = pos_k[qs].transpose(0, 2, 1).astype(bf)      # [q, d, t]
        A = pkt.reshape(TQ // 4, 4, DK, T)                 # [g, j, d, t]
        top = np.concatenate([A[:, 0], A[:, 2]], axis=2)   # [g, 64, 2T]
        bot = np.concatenate([A[:, 1], A[:, 3]], axis=2)
        pa = np.ascontiguousarray(np.concatenate([top, bot], axis=1))
        # mask: [m, s*32+j*8+b, t] = 1-mask[b, 16m+4s+j, t]
        mm = (1.0 - mask[:, qs, :].astype(np.float32)).transpose(1, 0, 2)
        mp = np.ascontiguousarray(
            mm.reshape(TQ // 16, 4, 4, B, T).reshape(TQ // 16, 128, T)
        ).astype(bf)
        qt = np.ascontiguousarray(q_in[:, qs, :].reshape(TOKQ, F).T).astype(bf)
        m = dict(shared)
        m["posk_p"] = pa
        m["mask_p"] = mp
        m["q_t"] = qt
        per_core.append(m)
    flags = dict(bv2_nz=bool(np.any(bv2)), bo_nz=bool(np.any(bo)))
    return per_core, flags


_PROGRAM_CACHE = {}


def kernel(**inputs):
    per_core, flags = make_core_inputs(inputs, CFG)
    key = (tuple(sorted(CFG.items())), tuple(sorted(flags.items())))
    if key not in _PROGRAM_CACHE:
        _PROGRAM_CACHE[key] = build_program(CFG, **flags)
    nc = _PROGRAM_CACHE[key]
    res = run_bass_kernel_spmd(nc, per_core, core_ids=list(range(NCORES)))
    outs = [res.results[c]["out"] for c in range(NCORES)]
    return np.concatenate(outs, axis=1).astype(np.float32)



# revision 47
# speedup vs baseline: 1.5661x; 1.5054x over previous
"""Cross multi-headed attention with relative position bias, sharded over 8 trn2 cores.

Sharding: query positions (T1=1024) split 128/core. pos_k (the 256MB tensor) is
indexed by query position, so each core streams a disjoint 1/8 slice exactly
once. K/V are computed from the full x on every core (cheap); outputs are
disjoint q-slices gathered on host.

Layout tricks:
  - pos_k is host-transposed/packed to (qgroup, dk+B, 2*T2) so it streams as
    the matmul moving operand (contraction dim dk on partitions) in 294KB DMAs,
    self-paced on the SP engine (a dedicated queue that blocks on pool bufs).
  - The mask penalty is folded into the Bmat matmul as 8 extra contraction
    rows (one-hot per batch x -1e4 penalty): masking costs zero vector ops and
    masked attn weights are exactly 0 after exp underflow.
  - Bmat comes out of its matmul in per-q tiles (32 bh-rows x T2); an
    SBUF->SBUF permute DMA regroups it into a resident 8MB SBUF buffer of
    per-(b,h) tiles (128q x T2) - no DRAM roundtrip - which an identity-matmul
    accumulates straight onto the QK^T scores in PSUM.
  - xn^T lives in a small ring (consumed per-chunk by the K/V projections),
    freeing the SBUF needed for the resident Bmat buffer.
  - LayerNorm gain/bias are folded into the projection weights on host;
    stats come from one bn_stats/bn_aggr pass; mean/rstd applied in natural
    layout (GPSIMD), then xn is PE-transposed for the projections.
  - Softmax denominators come from the exp activation's accum_out for free;
    no max-subtraction needed since scores are bounded (~N(0,2) after scale).
"""

import numpy as np
import ml_dtypes
from contextlib import ExitStack

import concourse.bass as bass
import concourse.tile as tile
import concourse.mybir as mybir
from concourse import bacc
from concourse.bass_utils import run_bass_kernel_spmd
from concourse.masks import make_identity

F32 = mybir.dt.float32
F32R = mybir.dt.float32r
BF16 = mybir.dt.bfloat16
FP8 = mybir.dt.float8e3

B, T, F, H, DK = 8, 1024, 256, 4, 64
NCORES = 8
TQ = T // NCORES          # query rows per core (128)
TOK = B * T               # tokens for K/V (8192)
TOKQ = B * TQ             # query tokens per core (1024)
NBH = B * H               # 32 (b,h) pairs
KAUG = DK + B             # bmat contraction rows: 64 qk + 8 mask-penalty
EPS = 1e-5
PEN = 1e4                 # mask penalty (score -= PEN/8 => exp underflows to 0)

CFG = dict()

AF = mybir.ActivationFunctionType
ALU = mybir.AluOpType
AXX = mybir.AxisListType


def build_program(cfg=CFG, bv2_nz=False, bo_nz=False):
    nc = bacc.Bacc("TRN2", target_bir_lowering=False, debug=False)

    # ---- DRAM I/O ----
    # x packed as 16 super-tiles: [st, tok%128, (tok//128 % 4)*F + f]
    x_d = nc.dram_tensor("xp", [TOK // 512, 128, 4 * F], BF16, kind="ExternalInput")
    qt_d = nc.dram_tensor("q_t", [F, TOKQ], BF16, kind="ExternalInput")
    # pos_k packed 4 q per tile on all 128 partitions (4KB lines):
    # rows 0:64 = d-rows of even local q, 64:128 = odd; cols 0:T = q-pair A,
    # T:2T = q-pair B.  [g, band*64+d, pair*T + t]
    posk_d = nc.dram_tensor("posk_p", [TQ // 4, 128, 2 * T], BF16,
                            kind="ExternalInput")
    # mask penalty operand: tile m covers groups 4m..4m+3 (16 q); row
    # s*32 + j*8 + b = (1-mask[b, q=16m+4s+j, t])
    maskp_d = nc.dram_tensor("mask_p", [TQ // 16, 128, T], BF16,
                             kind="ExternalInput")
    # stationary for the mask matmul: [s*32+j*8+b, j*32+b*H+h] = -PEN
    onehp_d = nc.dram_tensor("onehp", [128, 128], BF16, kind="ExternalInput")
    # DRAM bounce for the Bmat regroup: written per-group (contiguous, all 16
    # DMA ports), read back per-8-group batch into contiguous 32-partition
    # blocks of bmall (8 ports) - avoids the 4-partition SBUF->SBUF scatter.
    bounce_d = nc.dram_tensor("bm_bounce", [TQ // 4, 128, T], BF16,
                              kind="Internal")
    wq_d = nc.dram_tensor("wq", [F, F], BF16, kind="ExternalInput")
    wk_d = nc.dram_tensor("wk2", [F, F], BF16, kind="ExternalInput")
    wv_d = nc.dram_tensor("wv2", [F, F], BF16, kind="ExternalInput")
    wo_d = nc.dram_tensor("wo", [F, F], F32R, kind="ExternalInput")
    bq_d = nc.dram_tensor("bq_cols", [128, 2], F32, kind="ExternalInput")
    bk_d = nc.dram_tensor("bk2_cols", [128, 2], F32, kind="ExternalInput")
    bv_d = nc.dram_tensor("bv2_row", [1, F], BF16, kind="ExternalInput")
    bo_d = nc.dram_tensor("bo_row", [1, F], F32R, kind="ExternalInput")
    ones_d = nc.dram_tensor("ones_row", [1, 128], F32R, kind="ExternalInput")
    out_d = nc.dram_tensor("out", [B, TQ, F], F32, kind="ExternalOutput")

    with tile.TileContext(nc) as tc, ExitStack() as ctx:
        consts = ctx.enter_context(tc.tile_pool(name="consts", bufs=1))
        persist = ctx.enter_context(tc.tile_pool(name="persist", bufs=1))

        # ---- constants (loaded on sync before the posk stream starts) ----
        id_bf = consts.tile([128, 128], BF16)
        make_identity(nc, id_bf)
        id_f32 = consts.tile([128, 128], F32)
        make_identity(nc, id_f32)
        wq_sb = consts.tile([128, 2 * F], BF16)   # [f%128, (f//128)*F + n]
        wk_sb = consts.tile([128, 2 * F], BF16)
        wv_sb = consts.tile([128, 2 * F], BF16)
        wo_sb = consts.tile([64, 4 * F], F32R)    # [hd%64, (hd//64)*F + n]
        onehp_sb = consts.tile([128, 128], BF16)
        nc.sync.dma_start(onehp_sb[:], onehp_d[:])
        zrow = consts.tile([1, 128], BF16)
        nc.gpsimd.memset(zrow[:], 0.0)
        for w_sb, w_d in ((wq_sb, wq_d), (wk_sb, wk_d), (wv_sb, wv_d)):
            for fc in range(2):
                nc.sync.dma_start(w_sb[:, fc * F:(fc + 1) * F],
                                  w_d[fc * 128:(fc + 1) * 128, :])
        for ci in range(4):
            nc.sync.dma_start(wo_sb[:, ci * F:(ci + 1) * F],
                              wo_d[ci * 64:(ci + 1) * 64, :])
        bq_sb = consts.tile([128, 2], F32)
        nc.sync.dma_start(bq_sb[:], bq_d[:])
        bk_sb = consts.tile([128, 2], F32)
        nc.sync.dma_start(bk_sb[:], bk_d[:])
        if bv2_nz:
            bv_sb = consts.tile([1, F], BF16)
            nc.sync.dma_start(bv_sb[:], bv_d[:])
            ones_bf = consts.tile([1, 128], BF16)
            nc.gpsimd.memset(ones_bf[:], 1.0)
        if bo_nz:
            bo_sb = consts.tile([1, F], F32R)
            nc.sync.dma_start(bo_sb[:], bo_d[:])
            ones_r = consts.tile([1, 128], F32R)
            nc.sync.dma_start(ones_r[:], ones_d[:])
        eps_col = consts.tile([128, 1], F32)
        nc.gpsimd.memset(eps_col[:], EPS)

        # ---- persistent activations ----
        kt_sb = persist.tile([128, 2 * TOK], BF16)     # K^T: [n%128, (n//128)*TOK+tok]
        v_sb = persist.tile([128, (TOK // 128) * F], BF16)  # V: [tok%128,(tok//128)*F+hd]
        # rows 0:64 = Q^T [d, b*H*TQ + h*TQ + q]; rows 64:128 = copy of 0:64
        qq2 = persist.tile([128, TQ * NBH], BF16)
        # resident regrouped Bmat: [q, bh*T + t]  (8MB, 64KB/partition)
        bmall = persist.tile([128, NBH * T], BF16)

        NST = TOK // 512  # 16 super tiles

        # ================= Phase B: Q^T projection -> qq_sb =================
        with tc.tile_pool(name="qstage", bufs=1) as qstage, \
             tc.tile_pool(name="ps_q", bufs=2, space="PSUM") as ps_q:
            qt_tiles = []
            for fc in range(2):
                qt = qstage.tile([128, TOKQ], BF16, tag=f"qt{fc}")
                nc.scalar.dma_start(qt[:], qt_d[fc * 128:(fc + 1) * 128, :])
                qt_tiles.append(qt)
            qv = qq2[0:DK].rearrange("p (b h q) -> p b q h", h=H, q=TQ)
            for h in range(H):
                qp = ps_q.tile([DK, TOKQ], F32, tag="qp")
                for half in range(2):
                    for fc in range(2):
                        nc.tensor.matmul(
                            qp[:, half * 512:(half + 1) * 512],
                            wq_sb[:, fc * F + h * DK: fc * F + (h + 1) * DK],
                            qt_tiles[fc][:, half * 512:(half + 1) * 512],
                            start=(fc == 0), stop=(fc == 1))
                src = qp.rearrange("p (b q) -> p b q", b=B)
                bias = bq_sb[64 * (h % 2):64 * (h % 2) + 64, h // 2: h // 2 + 1]
                nc.scalar.activation(qv[:, :, :, h], src, AF.Identity, bias=bias)
            nc.gpsimd.dma_start(qq2[64:128, :], qq2[0:DK, :])

        # ===== Phases A+C+D: software-pipelined wavefront emission =====
        # Stages per index: S1(s)=stats, S2(s)=affine+transpose, S3(s)=K/V,
        # S4(qg)=bmat+staging+regroup. Emission is skewed so every engine's
        # queue holds ready work from the next iteration instead of blocking
        # on the tail of the current one.
        NGRP = 32
        with tc.tile_pool(name="poskst", bufs=4) as poskst, \
             tc.tile_pool(name="maskst", bufs=2) as maskst, \
             tc.tile_pool(name="bmstage", bufs=3) as bmstage, \
             tc.tile_pool(name="ps_bm", bufs=3, space="PSUM") as ps_bm:

            pk_tiles = {}
            mk_tiles = {}

            def s_pk(g):
                if not 0 <= g < NGRP:
                    return
                pk = poskst.tile([128, 2 * T], BF16, tag="pk")
                (nc.sync if g % 2 == 0 else nc.gpsimd).dma_start(pk[:], posk_d[g])
                pk_tiles[g] = pk

            def s_mk(m):
                if not 0 <= m < NGRP // 4:
                    return
                mk = maskst.tile([128, T], BF16, tag="mk")
                nc.scalar.dma_start(mk[:], maskp_d[m])
                mk_tiles[m] = mk

            def s4_bmat(g):
                if not 0 <= g < NGRP:
                    return
                pk = pk_tiles.pop(g)
                mk = mk_tiles[g // 4]
                strip = (g % 4) * 32
                qqe = qq2[0:DK].rearrange("p (b h q) -> p b h q", h=H, q=TQ)
                qqo = qq2[64:128].rearrange("p (b h q) -> p b h q", h=H, q=TQ)
                bs = bmstage.tile([128, T], BF16, tag="bs")
                for half in range(2):
                    hs = slice(half * 512, (half + 1) * 512)
                    bp = ps_bm.tile([128, 512], F32, tag="bp")
                    # mask penalty first: writes/sets has_written on the whole
                    # bank; the q-matmuls then accumulate on top.
                    nc.tensor.matmul(bp[:], onehp_sb[strip:strip + 32, :],
                                     mk[strip:strip + 32, hs],
                                     start=True, stop=False,
                                     tile_position=(strip, 0))
                    for j in range(4):
                        band = 64 * (j % 2)
                        lhs = (qqe if j % 2 == 0 else qqo)[:, :, :, g * 4 + j]
                        nc.tensor.matmul(
                            bp[32 * j:32 * (j + 1), :],
                            lhs,
                            pk[band:band + 64,
                               (j // 2) * T + half * 512:
                               (j // 2) * T + (half + 1) * 512],
                            start=False, stop=False,
                            tile_position=(band, 32 * j))
                    # zero-valued group closer spanning all 128 partitions
                    # (keeps the per-zero-region accumulation bookkeeping
                    # consistent; adds exact 0.0 to column 0).
                    nc.tensor.matmul(bp[:, 0:1], zrow[:], zrow[:, 0:1],
                                     start=False, stop=True)
                    if half == 0:
                        nc.vector.tensor_copy(bs[:, :512], bp[:])
                    else:
                        nc.scalar.activation(bs[:, 512:], bp[:], AF.Copy)
                if g % 4 == 3:
                    del mk_tiles[g // 4]
                # stage to DRAM (contiguous; regrouped on the way back)
                nc.scalar.dma_start(bounce_d[g], bs[:])

            def s5_fill(m):
                # read back 4 groups: DRAM [(g j), bh, t] -> bmall rows
                # 16m..16m+16 (contiguous partitions)
                src5 = bounce_d[4 * m:4 * (m + 1)].rearrange(
                    "g (j bh) t -> (g j) bh t", j=4)
                dst5 = bmall[16 * m:16 * (m + 1)].rearrange(
                    "p (bh t) -> p bh t", bh=NBH)
                nc.gpsimd.dma_start(dst5, src5)

            with tc.tile_pool(name="xstage", bufs=4) as xstage, \
                 tc.tile_pool(name="xcring", bufs=3) as xcring, \
                 tc.tile_pool(name="stats", bufs=3) as stats, \
                 tc.tile_pool(name="scratch", bufs=2) as scratch, \
                 tc.tile_pool(name="ps_a", bufs=2, space="PSUM") as ps_a, \
                 tc.tile_pool(name="ps_kv", bufs=3, space="PSUM") as ps_kv:

                xt_tiles = {}
                st_out = {}
                xc_tiles = {}

                def s0_load(s):
                    if not 0 <= s < NST:
                        return
                    xt = xstage.tile([128, 4 * F], BF16, tag="xt")
                    nc.sync.dma_start(xt[:], x_d[s])
                    xt_tiles[s] = xt

                def s1_stats(s):
                    if not 0 <= s < NST:
                        return
                    xt = xt_tiles[s]
                    GRP = 4
                    sums = stats.tile([128, GRP], F32, tag="sums")
                    sumsq = stats.tile([128, GRP], F32, tag="sumsq")
                    for c in range(4):
                        nc.vector.reduce_sum(sums[:, c:c + 1],
                                             xt[:, c * F:(c + 1) * F], axis=AXX.X)
                        sq = scratch.tile([128, F], BF16, tag="sq")
                        nc.scalar.activation(sq[:], xt[:, c * F:(c + 1) * F],
                                             AF.Square, accum_out=sumsq[:, c:c + 1])
                    negmu = stats.tile([128, GRP], F32, tag="negmu")
                    nc.vector.tensor_scalar(out=negmu[:], in0=sums[:], scalar1=-1.0 / F,
                                            scalar2=None, op0=ALU.mult)
                    msq = stats.tile([128, GRP], F32, tag="msq")
                    nc.vector.tensor_mul(msq[:], negmu[:], negmu[:])
                    var = stats.tile([128, GRP], F32, tag="var")
                    nc.vector.tensor_scalar(out=var[:], in0=sumsq[:], scalar1=1.0 / F,
                                            scalar2=None, op0=ALU.mult)
                    nc.vector.tensor_tensor(var[:], var[:], msq[:], op=ALU.subtract)
                    sig = stats.tile([128, GRP], F32, tag="sig")
                    nc.scalar.activation(sig[:], var[:], AF.Sqrt, bias=eps_col[:])
                    rsig = stats.tile([128, GRP], F32, tag="rsig")
                    nc.vector.reciprocal(rsig[:], sig[:])
                    cc = stats.tile([128, GRP], F32, tag="cc")
                    nc.vector.tensor_mul(cc[:], negmu[:], rsig[:])
                    st_out[s] = (rsig, cc)

                def s2_affine_t(s):
                    if not 0 <= s < NST:
                        return
                    xt = xt_tiles.pop(s)
                    rsig, cc = st_out.pop(s)
                    xn = scratch.tile([128, 4 * F], BF16, tag="xn")
                    for c in range(4):
                        nc.gpsimd.tensor_scalar(
                            out=xn[:, c * F:(c + 1) * F],
                            in0=xt[:, c * F:(c + 1) * F],
                            scalar1=rsig[:, c:c + 1], scalar2=cc[:, c:c + 1],
                            op0=ALU.mult, op1=ALU.add)
                    # xn^T chunk: [f%128, fc*512 + tok]
                    xc = xcring.tile([128, 2 * 512], BF16, tag="xc")
                    for fc in range(2):
                        tp = ps_a.tile([128, 512], BF16, tag="tp")
                        for c in range(4):
                            nc.tensor.transpose(
                                tp[:, c * 128:(c + 1) * 128],
                                xn[:, c * F + fc * 128: c * F + fc * 128 + 128],
                                id_bf[:])
                        nc.vector.tensor_copy(xc[:, fc * 512:(fc + 1) * 512], tp[:])
                    xc_tiles[s] = xc

                def s3_kv(s):
                    if not 0 <= s < NST:
                        return
                    xc = xc_tiles.pop(s)
                    for ncI in range(2):
                        kp = ps_kv.tile([128, 512], F32, tag="kv")
                        for fc in range(2):
                            nc.tensor.matmul(
                                kp[:],
                                wk_sb[:, fc * F + ncI * 128: fc * F + (ncI + 1) * 128],
                                xc[:, fc * 512:(fc + 1) * 512],
                                start=(fc == 0), stop=(fc == 1))
                        nc.scalar.activation(
                            kt_sb[:, ncI * TOK + s * 512: ncI * TOK + (s + 1) * 512],
                            kp[:], AF.Identity, bias=bk_sb[:, ncI:ncI + 1])
                    for gp in range(2):
                        vp = ps_kv.tile([128, 2 * F], F32, tag="kv")
                        for gi in range(2):
                            g4 = gp * 2 + gi
                            for fc in range(2):
                                nc.tensor.matmul(
                                    vp[:, gi * F:(gi + 1) * F],
                                    xc[:, fc * 512 + g4 * 128: fc * 512 + (g4 + 1) * 128],
                                    wv_sb[:, fc * F:(fc + 1) * F],
                                    start=(fc == 0), stop=(fc == 1 and not bv2_nz))
                            if bv2_nz:
                                nc.tensor.matmul(vp[:, gi * F:(gi + 1) * F],
                                                 ones_bf[:], bv_sb[:],
                                                 start=False, stop=True)
                        g0 = s * 4 + gp * 2
                        nc.vector.tensor_copy(v_sb[:, g0 * F:(g0 + 2) * F], vp[:])

                # pipeline fill + steady state over the AC stages; 2 bmat
                # groups retire per iteration so all 32 finish with the loop.
                s0_load(0)
                s0_load(1)
                s_mk(0)
                for g in range(4):
                    s_pk(g)
                for i in range(NST + 2):
                    s0_load(i + 2)
                    s_pk(2 * i + 4)
                    s_pk(2 * i + 5)
                    if i % 2 == 0:
                        s_mk(i // 2 + 1)
                    s1_stats(i)
                    s2_affine_t(i - 1)
                    s3_kv(i - 2)
                    s4_bmat(2 * (i - 2))
                    s4_bmat(2 * (i - 2) + 1)
                    if i >= 3 and (i - 3) % 2 == 0:
                        s5_fill((i - 3) // 2)

        # ================= Phase E: attention per (b, h), skewed =================
        with tc.tile_pool(name="attst", bufs=3) as attst, \
             tc.tile_pool(name="ps_s", bufs=2, space="PSUM") as ps_s, \
             tc.tile_pool(name="ps_at", bufs=2, space="PSUM") as ps_at, \
             tc.tile_pool(name="ps_small", bufs=2, space="PSUM") as ps_small, \
             tc.tile_pool(name="avout", bufs=2) as avout:
            qa0 = qq2[0:DK].rearrange("p (c q) -> p c q", c=NBH)
            qa1 = qq2[64:128].rearrange("p (c q) -> p c q", c=NBH)
            e_sp = {}
            e_at = {}
            e_att = {}
            avt_tiles = {}

            def e1_scores(bh):
                if not 0 <= bh < NBH:
                    return
                b, h = bh // H, bh % H
                lhsq = (qa0 if h % 2 == 0 else qa1)[:, bh, :]
                sp = ps_s.tile([128, T], F32, tag="sp")
                for half in range(2):
                    hs = slice(half * 512, (half + 1) * 512)
                    nc.tensor.matmul(
                        sp[:, hs], lhsq,
                        kt_sb[(h % 2) * 64:(h % 2) * 64 + 64,
                              (h // 2) * TOK + b * T + half * 512:
                              (h // 2) * TOK + b * T + (half + 1) * 512],
                        start=True, stop=False)
                    nc.tensor.matmul(
                        sp[:, hs], id_bf[:],
                        bmall[:, bh * T + half * 512: bh * T + (half + 1) * 512],
                        start=False, stop=True)
                e_sp[bh] = sp

            def e2_softmax(bh):
                if not 0 <= bh < NBH:
                    return
                sp = e_sp.pop(bh)
                attn = attst.tile([128, T], BF16, tag="attn")
                dn = attst.tile([128, 1], F32, tag="dn")
                nc.scalar.activation(attn[:], sp[:], AF.Exp,
                                     scale=1.0 / np.sqrt(DK),
                                     accum_out=dn[:])
                rn = attst.tile([128, 1], F32, tag="rn")
                nc.vector.reciprocal(rn[:], dn[:])
                e_att[bh] = (attn, rn)

            def e3_transpose(bh):
                if not 0 <= bh < NBH:
                    return
                attn, rn = e_att[bh]
                att = attst.tile([128, T], BF16, tag="att")  # attn^T
                tp2 = ps_at.tile([128, T], BF16, tag="tp2")
                for c in range(8):
                    nc.tensor.transpose(tp2[:, c * 128:(c + 1) * 128],
                                        attn[:, c * 128:(c + 1) * 128], id_bf[:])
                nc.vector.tensor_copy(att[:], tp2[:])
                e_at[bh] = att

            def e4_av(bh):
                if not 0 <= bh < NBH:
                    return
                b, h = bh // H, bh % H
                att = e_at.pop(bh)
                _, rn = e_att.pop(bh)
                if h == 0:
                    avt_tiles[b] = avout.tile([64, 4 * TQ], F32R, tag="avt",
                                              name="avt")
                avt = avt_tiles[b]
                avp = ps_small.tile([128, DK], F32, tag="small")
                for c in range(8):
                    nc.tensor.matmul(
                        avp[:], att[:, c * 128:(c + 1) * 128],
                        v_sb[:, (b * 8 + c) * F + h * DK:
                             (b * 8 + c) * F + (h + 1) * DK],
                        start=(c == 0), stop=(c == 7))
                av = avout.tile([128, DK], F32, tag="av")
                nc.vector.tensor_scalar(out=av[:], in0=avp[:], scalar1=rn[:],
                                        scalar2=None, op0=ALU.mult)
                atp = ps_small.tile([DK, 128], F32, tag="small")
                nc.tensor.transpose(atp[:], av[:], id_f32[:])
                nc.vector.tensor_copy(avt[:, h * TQ:(h + 1) * TQ], atp[:])
                if h == H - 1:
                    op = ps_small.tile([128, F], F32, tag="small")
                    for ci in range(4):
                        nc.tensor.matmul(
                            op[:], avt[:, ci * TQ:(ci + 1) * TQ],
                            wo_sb[:, ci * F:(ci + 1) * F],
                            start=(ci == 0), stop=(ci == 3 and not bo_nz))
                    if bo_nz:
                        nc.tensor.matmul(op[:], ones_r[:], bo_sb[:],
                                         start=False, stop=True)
                    ob = avout.tile([128, F], F32, tag="ob")
                    nc.vector.tensor_copy(ob[:], op[:])
                    nc.gpsimd.dma_start(out_d[b], ob[:])

            for i in range(NBH + 3):
                e1_scores(i)
                e2_softmax(i - 1)
                e3_transpose(i - 2)
                e4_av(i - 3)

    nc._dbg_names = {
        "kt": kt_sb.tensor.name, "v": v_sb.tensor.name,
        "qq": qq2.tensor.name, "bmall": bmall.tensor.name,
    }
    nc.compile()
    return nc


def make_core_inputs(inputs, cfg=CFG):
    """Host-side sharding/layout. Returns (per_core_maps, bias_flags)."""
    x = np.asarray(inputs["x"], np.float32)
    q_in = np.asarray(inputs["q_in"], np.float32)
    pos_k = np.asarray(inputs["pos_k"], np.float32)
    mask = np.asarray(inputs["mask"])
    ln_g = np.asarray(inputs["ln_g"], np.float32)
    ln_b = np.asarray(inputs["ln_b"], np.float32)
    Wq, bq = np.asarray(inputs["Wq"], np.float32), np.asarray(inputs["bq"], np.float32)
    Wk, bk = np.asarray(inputs["Wk"], np.float32), np.asarray(inputs["bk"], np.float32)
    Wv, bv = np.asarray(inputs["Wv"], np.float32), np.asarray(inputs["bv"], np.float32)
    Wo, bo = np.asarray(inputs["Wo"], np.float32), np.asarray(inputs["bo"], np.float32)

    bf = ml_dtypes.bfloat16
    Wk2 = ln_g[:, None] * Wk
    bk2 = ln_b @ Wk + bk
    Wv2 = ln_g[:, None] * Wv
    bv2 = ln_b @ Wv + bv

    xp = np.ascontiguousarray(
        x.reshape(TOK // 512, 4, 128, F).transpose(0, 2, 1, 3).reshape(
            TOK // 512, 128, 4 * F)).astype(bf)
    shared = {
        "xp": xp,
        "wq": Wq.astype(bf),
        "wk2": Wk2.astype(bf),
        "wv2": Wv2.astype(bf),
        "wo": Wo.astype(np.float32),
        "bq_cols": np.ascontiguousarray(bq.reshape(2, 128).T).astype(np.float32),
        "bk2_cols": np.ascontiguousarray(bk2.reshape(2, 128).T).astype(np.float32),
        "bv2_row": bv2.reshape(1, F).astype(bf),
        "bo_row": bo.reshape(1, F).astype(np.float32),
        "ones_row": np.ones((1, 128), np.float32),
    }
    # mask-matmul stationary: rows s*32+j*8+b (same for each strip s),
    # cols j*32 + b*H + h = -PEN
    op = np.zeros((4, 4, B, 128), np.float32)
    for j in range(4):
        for bb in range(B):
            op[:, j, bb, j * 32 + bb * H:j * 32 + (bb + 1) * H] = -PEN
    shared["onehp"] = np.ascontiguousarray(op.reshape(128, 128)).astype(bf)


    per_core = []
    for c in range(NCORES):
        qs = slice(c * TQ, (c + 1) * TQ)
        # posk: [g, band*64+d, pair*T+t]; band = local q parity, pair = q//2%2
        pkt = pos_k[qs].transpose(0, 2, 1).astype(bf)      # [q, d, t]
        A = pkt.reshape(TQ // 4, 4, DK, T)                 # [g, j, d, t]
        top = np.concatenate([A[:, 0], A[:, 2]], axis=2)   # [g, 64, 2T]
        bot = np.concatenate([A[:, 1], A[:, 3]], axis=2)
        pa = np.ascontiguousarray(np.concatenate([top, bot], axis=1))
        # mask: [m, s*32+j*8+b, t] = 1-mask[b, 16m+4s+j, t]
        mm = (1.0 - mask[:, qs, :].astype(np.float32)).transpose(1, 0, 2)
        mp = np.ascontiguousarray(
            mm.reshape(TQ // 16, 4, 4, B, T).reshape(TQ // 16, 128, T)
        ).astype(bf)
        qt = np.ascontiguousarray(q_in[:, qs, :].reshape(TOKQ, F).T).astype(bf)
        m = dict(shared)
        m["posk_p"] = pa
        m["mask_p"] = mp
        m["q_t"] = qt
        per_core.append(m)
    flags = dict(bv2_nz=bool(np.any(bv2)), bo_nz=bool(np.any(bo)))
    return per_core, flags


_PROGRAM_CACHE = {}


def kernel(**inputs):
    per_core, flags = make_core_inputs(inputs, CFG)
    key = (tuple(sorted(CFG.items())), tuple(sorted(flags.items())))
    if key not in _PROGRAM_CACHE:
        _PROGRAM_CACHE[key] = build_program(CFG, **flags)
    nc = _PROGRAM_CACHE[key]
    res = run_bass_kernel_spmd(nc, per_core, core_ids=list(range(NCORES)))
    outs = [res.results[c]["out"] for c in range(NCORES)]
    return np.concatenate(outs, axis=1).astype(np.float32)

